# revision 16
# baseline (speedup 1.0000x reference)
"""Trainium2 Bass kernel for nn_ArcEmbedding (embedding lookup + 3-axis RoPE).

Reference computation (per token t in batch b):
    e = emb_table[id]                       # [768]
    theta = [xn*invf, yn*invf, tn*invf]     # [384], xn = x/max(max_b(x),1) etc
    out[0:384]   = e[0:384]*cos(theta) - e[384:768]*sin(theta)
    out[384:768] = e[384:768]*cos(theta) + e[0:384]*sin(theta)

Kernel strategy (data-parallel over batch, 4 batches per NeuronCore, 8 cores):
  Polar refactor: with e1=e[0:384], e2=e[384:768],
      r_s  = sign(e1)*sqrt(e1^2+e2^2)
      phi0 = atan(e2/e1) in (-pi/2, pi/2)
      out[0:384]   = r_s * cos(phi0 + theta) = r_s * sin(pi/2 - phi0 - theta)
      out[384:768] = r_s * sin(phi0 + theta)
  Per 128-token tile:
    - one-hot ids arrive pre-encoded from the host (same bytes as int ids)
    - psi_ext[128,768] = onehot @ [phi||-phi] table accumulated with the
      angle outer-product (x/mx, y/my, t_norm, ones rows against inv-freq
      rows) -- the phase addition happens inside the PE PSUM accumulation
    - r gather: rg[128,384] = onehot @ r_s table (same stationary weights)
    - ONE ACT Sin over [128,768] PSUM -> sin and cos halves simultaneously
      (all args stay within ACT Sin's valid domain [-3.797, 3.797])
    - final amplitude multiplies on DVE (2x bf16), casts split DVE/ACT
    - bf16 output tile -> HWDGE DMA to DRAM (host upcasts to f32)
  Tiles are processed in pairs to amortize per-instruction overheads.
"""

import numpy as np

B, S, H, V = 32, 4096, 768, 64
P = 128
NCORES = 8
BPC = B // NCORES            # batches per core
NT = S // P                  # 128-token tiles per batch
NPAIR = NT // 2              # tile pairs per batch
HALF = H // 2                # 384
DA = HALF // 3               # 128 freqs per axis
ROPE_BASE = 10000.0

_INVF = (1.0 / (ROPE_BASE ** (np.arange(DA, dtype=np.float64) / DA))).astype(
    np.float32
)
_TNORM = (np.arange(S, dtype=np.float64) / (S - 1)).astype(np.float32)

_COMPILED = {}
LAST_RESULTS = None


def _build_program():
    import concourse.bacc as bacc
    import concourse.mybir as mybir
    import concourse.tile as tile

    f32 = mybir.dt.float32
    bf16 = mybir.dt.bfloat16
    AF = mybir.ActivationFunctionType
    ALU = mybir.AluOpType

    nc = bacc.Bacc("TRN2", target_bir_lowering=False, debug=False)

    ids_d = nc.dram_tensor("oh", [BPC, V, S], bf16, kind="ExternalInput")
    xyt_d = nc.dram_tensor("xyt", [BPC, 4, S], bf16, kind="ExternalInput")
    xymax_d = nc.dram_tensor("xymax", [BPC, P, 2 * NT], bf16, kind="ExternalInput")
    emb_d = nc.dram_tensor("emb", [V, H], f32, kind="ExternalInput")
    rtail_d = nc.dram_tensor("rhs_tail", [4, H], bf16, kind="ExternalInput")
    ident_d = nc.dram_tensor("ident", [P, P], f32, kind="ExternalInput")
    out_d = nc.dram_tensor("out", [BPC, S, H], bf16, kind="ExternalOutput")

    with tile.TileContext(nc) as tc:
        with (
            tc.tile_pool(name="const", bufs=1) as cpool,
            tc.tile_pool(name="batch", bufs=3) as bpool,
            tc.tile_pool(name="work", bufs=3) as wpool,
            tc.tile_pool(name="psum", bufs=2, space="PSUM") as ppool,
            tc.tile_pool(name="psum1", bufs=1, space="PSUM") as ppool1,
        ):
            # ---------------- one-time setup ----------------
            emb_sb = cpool.tile([V, H], f32)
            nc.sync.dma_start(out=emb_sb[:], in_=emb_d[:])
            ident_t = cpool.tile([P, P], f32)
            nc.sync.dma_start(out=ident_t[:], in_=ident_d[:])

            rhs_psi = cpool.tile([68, H], bf16)
            nc.vector.memset(rhs_psi[:], 0.0)
            nc.sync.dma_start(out=rhs_psi[64:68, :], in_=rtail_d[:])
            rhs_r = cpool.tile([68, HALF], bf16)
            nc.vector.memset(rhs_r[:], 0.0)

            e1 = emb_sb[:, 0:HALF]
            e2 = emb_sb[:, HALF:H]
            sqb = cpool.tile([V, H], f32)
            nc.scalar.activation(out=sqb[:], in_=emb_sb[:], func=AF.Square)
            ssum = cpool.tile([V, HALF], f32)
            nc.vector.tensor_tensor(
                out=ssum[:], in0=sqb[:, 0:HALF], in1=sqb[:, HALF:H], op=ALU.add
            )
            rmag = cpool.tile([V, HALF], f32)
            nc.scalar.activation(out=rmag[:], in_=ssum[:], func=AF.Sqrt)
            neg = cpool.tile([V, HALF], f32)
            nc.vector.tensor_scalar(
                out=neg[:], in0=e1, scalar1=0.0, scalar2=None, op0=ALU.is_lt
            )
            sgn = cpool.tile([V, HALF], f32)
            nc.vector.tensor_scalar(
                out=sgn[:], in0=neg[:], scalar1=-2.0, scalar2=1.0,
                op0=ALU.mult, op1=ALU.add,
            )
            rsg = cpool.tile([V, HALF], f32)
            nc.vector.tensor_tensor(out=rsg[:], in0=rmag[:], in1=sgn[:], op=ALU.mult)
            nc.vector.tensor_copy(out=rhs_r[0:V, :], in_=rsg[:])

            einv = cpool.tile([V, HALF], f32)
            nc.vector.reciprocal(out=einv[:], in_=e1)
            quo = cpool.tile([V, HALF], f32)
            nc.vector.tensor_tensor(out=quo[:], in0=e2, in1=einv[:], op=ALU.mult)
            phi = cpool.tile([V, HALF], f32)
            nc.scalar.activation(out=phi[:], in_=quo[:], func=AF.Arctan)
            nc.vector.tensor_copy(out=rhs_psi[0:V, 0:HALF], in_=phi[:])
            nc.vector.tensor_scalar(
                out=rhs_psi[0:V, HALF:H], in0=phi[:], scalar1=-1.0, scalar2=None,
                op0=ALU.mult,
            )

            # ---------------- per batch ----------------
            for b in range(BPC):
                xyt_t = bpool.tile([4, S], bf16, tag="xyt")
                nc.sync.dma_start(out=xyt_t[:], in_=xyt_d[b])
                mxin = bpool.tile([P, 2 * NT], bf16, tag="mxin")
                nc.sync.dma_start(out=mxin[:], in_=xymax_d[b])
                mx2 = bpool.tile([P, 2], f32, tag="mx2")
                nc.vector.tensor_reduce(
                    out=mx2[:, 0:1], in_=mxin[:, 0:NT],
                    axis=mybir.AxisListType.X, op=ALU.max,
                )
                nc.vector.tensor_reduce(
                    out=mx2[:, 1:2], in_=mxin[:, NT:2 * NT],
                    axis=mybir.AxisListType.X, op=ALU.max,
                )
                pmx = ppool1.tile([2, P], f32, tag="pmx")
                nc.tensor.transpose(out=pmx[:], in_=mx2[:], identity=ident_t[:])
                stg = bpool.tile([2, 4], f32, tag="stg")
                nc.vector.tensor_reduce(
                    out=stg[:, 0:1], in_=pmx[:],
                    axis=mybir.AxisListType.X, op=ALU.max,
                )
                nc.vector.tensor_scalar(
                    out=stg[:, 1:2], in0=stg[:, 0:1], scalar1=1.0,
                    scalar2=None, op0=ALU.max,
                )
                nc.vector.reciprocal(out=stg[:, 2:3], in_=stg[:, 1:2])
                # Pre-scale the x/y rows by 1/mx, 1/my (partitions 0-1),
                # then DMA them across partitions into the lhsT tiles.
                xys = bpool.tile([2, S], bf16, tag="xys")
                nc.vector.tensor_scalar(
                    out=xys[:], in0=xyt_t[0:2, :], scalar1=stg[:, 2:3],
                    scalar2=None, op0=ALU.mult,
                )
                L = bpool.tile([68, S], bf16, tag="bigL")
                nc.sync.dma_start(out=L[0:V, :], in_=ids_d[b])
                nc.sync.dma_start(out=L[64:66, :], in_=xys[:])
                nc.sync.dma_start(out=L[66:68, :], in_=xyt_d[b, 2:4])

                for jp in range(NPAIR):
                    w0 = jp * 2 * P           # pair start column
                    # pair-contiguous staging:
                    #   sc2  = [sinA | sinB | cosA | cosB]   each 384 wide
                    #   rsb2 = [rA | rB]
                    #   ot   = [loA | loB | hiA | hiB]
                    sc2 = wpool.tile([P, 4 * HALF], bf16, tag="sc2")
                    rsb2 = wpool.tile([P, 2 * HALF], bf16, tag="rsb2")
                    for k in range(2):
                        wk = w0 + k * P
                        psi = ppool.tile([P, H], f32, tag="psi")
                        nc.tensor.matmul(
                            psi[:, 0:512], L[:, wk:wk + P], rhs_psi[:, 0:512],
                            start=True, stop=True,
                        )
                        nc.tensor.matmul(
                            psi[:, 512:H], L[:, wk:wk + P], rhs_psi[:, 512:H],
                            start=True, stop=True,
                        )
                        rg = ppool.tile([P, HALF], f32, tag="rg")
                        nc.tensor.matmul(
                            rg[:], L[:, wk:wk + P], rhs_r[:], start=True,
                            stop=True,
                        )
                        # sin half -> cols [k*384, k*384+384),
                        # cos half -> cols [768 + k*384, ...)
                        nc.scalar.activation(
                            out=sc2[:].rearrange(
                                "p (hs kk c) -> p hs kk c", hs=2, kk=2
                            )[:, :, k, :],
                            in_=psi[:].rearrange("p (a c) -> p a c", a=2, c=HALF),
                            func=AF.Sin,
                        )
                        if jp % 8 in (1, 4, 6) and k == 1:
                            # balance: scalar engine takes 1 of 4 casts
                            nc.scalar.copy(
                                out=rsb2[:, k * HALF:(k + 1) * HALF], in_=rg[:]
                            )
                        else:
                            nc.vector.tensor_copy(
                                out=rsb2[:, k * HALF:(k + 1) * HALF], in_=rg[:]
                            )
                    ot = wpool.tile([P, 4 * HALF], bf16, tag="ot")
                    # lo halves = r_s * cos  (contiguous [P, 768] at 2x)
                    nc.vector.tensor_tensor(
                        out=ot[:, 0:2 * HALF], in0=rsb2[:],
                        in1=sc2[:, 2 * HALF:4 * HALF], op=ALU.mult,
                    )
                    # hi halves = r_s * sin (also DVE: GPSIMD elementwise
                    # work steals the shared SBUF port and halves DVE)
                    nc.vector.tensor_tensor(
                        out=ot[:, 2 * HALF:4 * HALF], in0=rsb2[:],
                        in1=sc2[:, 0:2 * HALF], op=ALU.mult,
                    )
                    nc.sync.dma_start(
                        out=out_d[b, w0:w0 + 2 * P, :].rearrange(
                            "(k p) (hs c) -> p hs k c", k=2, c=HALF
                        ),
                        in_=ot[:].rearrange(
                            "p (hs k c) -> p hs k c", hs=2, k=2
                        ),
                    )

    nc.compile()
    return nc


def _host_inputs(input_ids, coords, emb_table):
    import ml_dtypes

    bf16 = ml_dtypes.bfloat16
    ids = np.asarray(input_ids).astype(np.float32)          # [B, S]
    xy = np.asarray(coords).astype(np.float32)              # [B, S, 2]
    emb = np.asarray(emb_table).astype(np.float32)          # [V, H]

    ident = np.eye(P, dtype=np.float32)
    rtail = np.zeros((4, H), dtype=np.float32)
    rtail[0, 0:DA] = _INVF                                   # x row, sin half
    rtail[0, HALF:HALF + DA] = -_INVF                        # x row, cos half
    rtail[1, DA:2 * DA] = _INVF                              # y row, sin half
    rtail[1, HALF + DA:HALF + 2 * DA] = -_INVF               # y row, cos half
    rtail[2, 2 * DA:HALF] = _INVF                            # t row, sin half
    rtail[2, HALF + 2 * DA:H] = -_INVF                       # t row, cos half
    rtail[3, HALF:H] = np.pi / 2                             # ones row, cos half
    rtail = rtail.astype(bf16)

    in_maps = []
    for c in range(NCORES):
        bs = slice(c * BPC, (c + 1) * BPC)
        oh = (
            ids[bs][:, None, :] == np.arange(V, dtype=np.float32)[None, :, None]
        ).astype(bf16)                                       # [BPC, V, S]
        xyt = np.empty((BPC, 4, S), dtype=np.float32)
        xyt[:, 0, :] = xy[bs, :, 0]
        xyt[:, 1, :] = xy[bs, :, 1]
        xyt[:, 2, :] = _TNORM[None, :]
        xyt[:, 3, :] = 1.0
        xymax = np.empty((BPC, P, 2 * NT), dtype=np.float32)
        xymax[:, :, 0:NT] = xy[bs, :, 0].reshape(BPC, NT, P).transpose(0, 2, 1)
        xymax[:, :, NT:2 * NT] = (
            xy[bs, :, 1].reshape(BPC, NT, P).transpose(0, 2, 1)
        )
        in_maps.append(
            {
                "oh": oh,
                "xyt": xyt.astype(bf16),
                "xymax": xymax.astype(bf16),
                "emb": emb,
                "rhs_tail": rtail,
                "ident": ident,
            }
        )
    return in_maps


def kernel(input_ids, coords, emb_table):
    global LAST_RESULTS
    from concourse.bass_utils import run_bass_kernel_spmd

    if "nc" not in _COMPILED:
        _COMPILED["nc"] = _build_program()
    nc = _COMPILED["nc"]

    in_maps = _host_inputs(input_ids, coords, emb_table)
    res = run_bass_kernel_spmd(nc, in_maps, core_ids=list(range(NCORES)))
    LAST_RESULTS = res
    out = np.concatenate(
        [r["out"].astype(np.float32) for r in res.results], axis=0
    )
    return out
